# revision 70
# baseline (speedup 1.0000x reference)
"""GNN message-passing NodeBlock kernel for 8 Trainium2 NeuronCores.

Problem:
    agg_a = segment_sum(edata_a, conn_a[1], 100000)   # [N, 64]
    agg_b = segment_sum(edata_b, conn_b[1], 100000)   # [N, 64]
    out   = concat([agg_a, agg_b, vdata], 1) @ W + b  # [N, 128]

Sharding: edges are sharded BY RECEIVER RANGE -- core c owns nodes
[c*12544, (c+1)*12544) and receives exactly the edges targeting them, so each
core computes its slice of the aggregation completely locally; no collective.

Scatter: edges are packed host-side into 128-slot tiles. Each tile's edges
all target a sliding 32-node window [q, q+32) whose base q is chosen by a
greedy scheduler shared across all 8 cores (q = min over cores of the next
unplaced edge, windows clipped to 512-node psum blocks). This keeps tiles
~96% full (vs ~86% for aligned windows). A tile is scattered into its window
by a one-hot selection matrix (is_equal of a 32-wide iota row vs the edge's
relative index, built on DVE) via one PE matmul accumulated into PSUM with
start=False; a DVE memset zero-initializes each psum block first (windows
overlap, so no matmul can own the start, and 0+x is correct whatever the
stale has_written state). Edge features travel as bf16 (quantization error
~2^-9 relative, final rel err ~4e-3 vs the 2e-2 gate). Type-a tiles write
psum partitions 0:64, type-b 64:128 and the a/b matmul streams interleave so
adjacent MMs land in different PE column groups. The dense updater runs as
bf16 matmuls over 512-node column blocks on the transposed layout
(out^T = W^T x^T) right after each block's scatter; outputs staged as bf16
and cast to fp32 on the host. A burst of zero-weight matmuls at kernel start
keeps PE busy through the DMA preroll so HAM reaches the 2.4 GHz state
before real tiles arrive.

SPMD: one program for all 8 cores; the (block, window-base) schedule is
identical across cores by construction, only the packed edge data differs.
Padding slots carry rel=-1 (matches no iota column) and zero data.

Perf notes (measured): per-scatter-MM ~42-53 ns (LDWEIGHTS-floor bound,
identical for bf16/fp8/64-or-128-col weights, so fp8 packing buys nothing);
DMA ~33.5 MB/core across only TWO hw DGE rings (sync + scalar); DVE sel
builds run at 1 elem/cycle because the iota/rel operands are broadcast APs.
Engine busy at 130 us: PE ~70 us, DVE ~70 us, DMA queues ~70%.
"""
import numpy as np
import ml_dtypes

import concourse.bass as bass
import concourse.tile as tile
from concourse import mybir
from concourse.bass_utils import run_bass_kernel_spmd
from concourse.vector_clock import ScopedClock

BF16 = ml_dtypes.bfloat16

N_NODES = 100000
N_EDGES = 800000
D_EDGE = 64
D_NODE = 128
D_OUT = 128
N_CORES = 8
NPC = 12544                # nodes per core
BLK = 512                  # psum block columns
N_BLKS = (NPC + BLK - 1) // BLK  # 25 (last block has 256 cols)
WSPAN = 32                 # sliding window span (sel columns per tile)
OUT_CHUNK = 2              # blocks per outT store

# ---------------------------------------------------------------------------
# compat patches for this container's walrus build
# ---------------------------------------------------------------------------

_MAX_WAITS = 1


def _patched_drain_and_barrier(self, tick_clock, wait_clock):
    nc = self.nc
    probe = nc.sync.nop(nofuse=True, hint="tile_drain_wait0")
    wait_clock.add_sem_waits(
        probe.ins, ScopedClock({None: tick_clock.global_clock})
    )
    si = probe.ins.sync_info
    waits = list(si.on_wait) if si is not None and si.on_wait else []
    if len(waits) > _MAX_WAITS:
        si.on_wait = waits[:_MAX_WAITS]
        for k in range(_MAX_WAITS, len(waits), _MAX_WAITS):
            n = nc.sync.nop(nofuse=True, hint=f"tile_drain_wait{k}")
            n.ins.sync_info = mybir.SyncInfo(
                on_wait=waits[k : k + _MAX_WAITS], on_update=[]
            )
    drain_inst = nc.sync.drain()
    wait_clock.add_sem_waits(
        drain_inst.ins, ScopedClock({None: tick_clock.global_clock})
    )
    dsi = drain_inst.ins.sync_info
    if dsi is not None and dsi.on_wait and len(dsi.on_wait) > _MAX_WAITS:
        dsi.on_wait = []
    nc.all_engine_barrier()
    assert self.sems is not None
    popped = nc._tile_sem_poison_stack.pop()
    assert popped is self._sem_poison
    nc.clear_and_free_semaphores(list(self.sems.allocated().values()))
    nc.all_engine_barrier()


def _split_multi_waits(nc):
    """This walrus build accepts one sync-wait per TPB instruction; move
    extra waits onto preceding same-engine NOPs."""
    for fn in nc.m.functions:
        for blk in fn.blocks:
            out = []
            changed = False
            for inst in blk.instructions:
                si = inst.sync_info
                if si is not None and si.on_wait and len(si.on_wait) > 1:
                    waits = list(si.on_wait)
                    for j, w in enumerate(waits[:-1]):
                        nop = mybir.InstNoOp(
                            name=f"{inst.name}_xw{j}", ins=[], outs=[]
                        )
                        nop.engine = inst.engine
                        nop.sync_info = mybir.SyncInfo(
                            on_wait=[w], on_update=[]
                        )
                        out.append(nop)
                    si.on_wait = [waits[-1]]
                    changed = True
                out.append(inst)
            if changed:
                blk.instructions = out


def _install_ntff_hook_shim():
    import sys
    import types

    if "antenv.axon_hooks" in sys.modules:
        return
    mod = types.ModuleType("antenv.axon_hooks")
    _hook = [None]
    mod.set_axon_ntff_profile_hook = lambda h: _hook.__setitem__(0, h)
    mod.get_axon_ntff_profile_hook = lambda: _hook[0]
    sys.modules["antenv.axon_hooks"] = mod
    try:
        import antenv

        antenv.axon_hooks = mod
    except ImportError:
        pass
    try:
        from trn_agent_boot.trn_boot import _ntff_profile_via_ctypes

        mod.set_axon_ntff_profile_hook(
            _ntff_profile_via_ctypes("/opt/axon/libaxon_pjrt.so")
        )
    except Exception:
        pass


tile.TileContext._drain_and_barrier = _patched_drain_and_barrier
_install_ntff_hook_shim()

# ---------------------------------------------------------------------------
# host-side sharding / packing
# ---------------------------------------------------------------------------


def _build_schedule(recv):
    """Greedy sliding-window tile schedule shared across cores.

    Returns (tiles, core_fill): tiles = [(block j, col offset c0), ...];
    core_fill[c] = list (per tile) of np arrays of edge ids.
    """
    order = np.argsort(recv, kind="stable")
    snodes = recv[order]
    bounds = np.searchsorted(snodes, np.arange(N_CORES + 1) * NPC)
    arrs = []
    for c in range(N_CORES):
        sl = slice(bounds[c], bounds[c + 1])
        arrs.append((snodes[sl] - c * NPC, order[sl]))
    pos = [0] * N_CORES
    n = [len(a[0]) for a in arrs]
    tiles = []
    core_fill = [[] for _ in range(N_CORES)]
    while True:
        q = NPC
        for c in range(N_CORES):
            if pos[c] < n[c]:
                q = min(q, int(arrs[c][0][pos[c]]))
        if q == NPC:
            break
        j = q // BLK
        wend = min(q + WSPAN, (j + 1) * BLK, NPC)
        tiles.append((j, q - j * BLK))
        for c in range(N_CORES):
            rel, eids = arrs[c]
            p = pos[c]
            e = min(int(np.searchsorted(rel, wend, side="left")), p + 128)
            core_fill[c].append(eids[p:e])
            pos[c] = e
    return tiles, core_fill


def _preprocess(vdata, edata_a, edata_b, conn_a, conn_b, W_mat, b_vec):
    recv_a = np.asarray(conn_a[1]).astype(np.int64)
    recv_b = np.asarray(conn_b[1]).astype(np.int64)

    tiles_a, fill_a = _build_schedule(recv_a)
    tiles_b, fill_b = _build_schedule(recv_b)

    # per-block tile index ranges (a and b separately; both block-sorted)
    def per_block(tiles):
        cnt = [0] * N_BLKS
        for j, _ in tiles:
            cnt[j] += 1
        starts = np.zeros(N_BLKS + 1, dtype=np.int64)
        np.cumsum(cnt, out=starts[1:])
        return cnt, starts

    cnt_a, st_a = per_block(tiles_a)
    cnt_b, st_b = per_block(tiles_b)
    T_tot = len(tiles_a) + len(tiles_b)

    # stream slot of tile k: block j holds [a tiles | b tiles]
    blk_base = np.zeros(N_BLKS + 1, dtype=np.int64)
    np.cumsum([cnt_a[j] + cnt_b[j] for j in range(N_BLKS)], out=blk_base[1:])
    slot_of_a = [
        int(blk_base[j] + (k - st_a[j]))
        for k, (j, _) in enumerate(tiles_a)
    ]
    slot_of_b = [
        int(blk_base[j] + cnt_a[j] + (k - st_b[j]))
        for k, (j, _) in enumerate(tiles_b)
    ]

    e16_a = np.asarray(edata_a).astype(BF16)
    e16_b = np.asarray(edata_b).astype(BF16)

    vdata = np.asarray(vdata)
    NTOT = NPC * N_CORES
    vpad = np.zeros((NTOT, D_NODE), dtype=BF16)
    vpad[:N_NODES] = vdata.astype(BF16)

    iota = np.ascontiguousarray(
        np.broadcast_to(np.arange(WSPAN, dtype=np.float32), (128, WSPAN))
    ).astype(BF16)
    Wf = np.ascontiguousarray(np.asarray(W_mat)).astype(BF16)
    bf = np.asarray(b_vec).astype(np.float32).reshape(D_OUT, 1)

    in_maps = []
    for c in range(N_CORES):
        slot_eid = np.full(T_tot * 128, -1, dtype=np.int64)
        slot_rel = np.full(T_tot * 128, -1.0, dtype=np.float32)
        slot_is_a = np.zeros(T_tot * 128, dtype=bool)
        for tiles, fill, slots, is_a in (
            (tiles_a, fill_a[c], slot_of_a, True),
            (tiles_b, fill_b[c], slot_of_b, False),
        ):
            recv = recv_a if is_a else recv_b
            for k, (j, c0) in enumerate(tiles):
                eids = fill[k]
                if len(eids) == 0:
                    continue
                s0 = slots[k] * 128
                slot_eid[s0 : s0 + len(eids)] = eids
                slot_is_a[s0 : s0 + len(eids)] = is_a
                qabs = c * NPC + j * BLK + c0
                slot_rel[s0 : s0 + len(eids)] = (recv[eids] - qabs).astype(
                    np.float32
                )
        idx = np.maximum(slot_eid, 0)
        gath = np.where(slot_is_a[:, None], e16_a[idx], e16_b[idx])
        gath[slot_eid < 0] = 0
        eh16 = np.ascontiguousarray(
            gath.reshape(T_tot, 128, 64).transpose(1, 0, 2)
        )  # [slot, tile, feat] bf16
        rel = np.ascontiguousarray(
            slot_rel.reshape(T_tot, 128).T.astype(BF16)
        )  # [128, T]
        vT = np.ascontiguousarray(vpad[c * NPC : (c + 1) * NPC].T)  # [128,NPC]
        in_maps.append(
            {"eh16": eh16, "rel": rel, "vT": vT, "Wd": Wf, "bd": bf,
             "iota": iota}
        )

    sched = (
        tuple((j, c0) for j, c0 in tiles_a),
        tuple((j, c0) for j, c0 in tiles_b),
    )
    return in_maps, sched


# ---------------------------------------------------------------------------
# device kernel
# ---------------------------------------------------------------------------

_NC_CACHE = {}


def _build(sched):
    tiles_a, tiles_b = sched
    f32 = mybir.dt.float32
    bf16 = mybir.dt.bfloat16

    blk_a = [[] for _ in range(N_BLKS)]  # c0 lists per block
    blk_b = [[] for _ in range(N_BLKS)]
    for j, c0 in tiles_a:
        blk_a[j].append(c0)
    for j, c0 in tiles_b:
        blk_b[j].append(c0)
    blk_na = [len(x) for x in blk_a]
    blk_nb = [len(x) for x in blk_b]
    blk_tot = [a + b for a, b in zip(blk_na, blk_nb)]
    max_blk = max(blk_tot)
    max_half = max(max(blk_na), max(blk_nb))
    T_tot = sum(blk_tot)

    nc = bass.Bass(trn_type="TRN2")
    eh16_d = nc.dram_tensor("eh16", [128, T_tot, 64], bf16, kind="ExternalInput")
    rel_d = nc.dram_tensor("rel", [128, T_tot], bf16, kind="ExternalInput")
    vT_d = nc.dram_tensor("vT", [128, NPC], bf16, kind="ExternalInput")
    W_d = nc.dram_tensor("Wd", [2 * D_NODE, D_OUT], bf16, kind="ExternalInput")
    b_d = nc.dram_tensor("bd", [D_OUT, 1], f32, kind="ExternalInput")
    iota_d = nc.dram_tensor("iota", [128, WSPAN], bf16, kind="ExternalInput")
    outT_d = nc.dram_tensor("outT", [128, NPC], bf16, kind="ExternalOutput")

    with tile.TileContext(nc) as tc:
        with (
            tc.tile_pool(name="consts", bufs=1) as cb,
            tc.tile_pool(name="x0", bufs=3) as x0p,
            tc.tile_pool(name="edges", bufs=6) as ep,
            tc.tile_pool(name="sel", bufs=6) as sp,
            tc.tile_pool(name="relp", bufs=6) as rp,
            tc.tile_pool(name="out", bufs=3) as op,
            tc.tile_pool(name="psum1", bufs=5, space="PSUM") as pp1,
            tc.tile_pool(name="psum2", bufs=2, space="PSUM") as pp2,
            tc.tile_pool(name="psumw", bufs=1, space="PSUM") as pwp,
        ):
            iota_sb = cb.tile([128, WSPAN], bf16)
            nc.scalar.dma_start(iota_sb[:], iota_d[:, :])
            w0_sb = cb.tile([128, D_OUT], bf16, tag="w0")
            nc.scalar.dma_start(w0_sb[:], W_d[0:128, :])
            w1_sb = cb.tile([128, D_OUT], bf16, tag="w1")
            nc.scalar.dma_start(w1_sb[:], W_d[128:256, :])
            b_sb = cb.tile([D_OUT, 1], f32, tag="b")
            nc.scalar.dma_start(b_sb[:], b_d[:, :])
            zer_sb = cb.tile([128, BLK], bf16, tag="zer")
            nc.vector.memset(zer_sb[:], 0.0)
            vt_sb = cb.tile([128, NPC], bf16, tag="vt")

            # HAM warm-up: keep PE busy during the DMA preroll so the
            # first real tiles run at 2.4 GHz (no data dependencies)
            wps = pwp.tile([128, BLK], f32, tag="pw")
            for _ in range(20):
                nc.tensor.matmul(
                    out=wps[:, :], lhsT=zer_sb[:, 0:128], rhs=zer_sb[:, :],
                    start=True, stop=True, skip_group_check=True,
                )

            off = 0
            pend = None
            emit_mms, emit_finish = _make_updater(
                nc, tc, (pp2, op),
                (w0_sb, w1_sb, b_sb, vt_sb, outT_d, NPC),
            )
            for j in range(N_BLKS):
                cols_blk = min(BLK, NPC - j * BLK)
                n_blk = blk_tot[j]
                na_b, nb_b = blk_na[j], blk_nb[j]

                # rel chunk rides with this block's edges (no head-of-line
                # blocking from one big upfront transfer)
                # only two HW DMA rings exist (sync + scalar); split the
                # bulk edge halves across both so they land in parallel.
                # rel rides first on the sync ring (sel needs it first).
                rel_t = rp.tile([128, max_blk], bf16, tag="relt")
                nc.sync.dma_start(
                    rel_t[:, :n_blk], rel_d[:, off : off + n_blk]
                )
                et16a = ep.tile([128, max_half * 64], bf16, tag="et16a")
                nc.sync.dma_start(
                    et16a[:, : na_b * 64], eh16_d[:, off : off + na_b, :]
                )
                et16b = ep.tile([128, max_half * 64], bf16, tag="et16b")
                nc.sync.dma_start(
                    et16b[:, : nb_b * 64],
                    eh16_d[:, off + na_b : off + n_blk, :],
                )
                # vT woven in 16 chunks; chunk j covers block j's updater
                # columns in time
                if j < 16:
                    vc0 = j * (NPC // 16)
                    vc1 = NPC if j == 15 else (j + 1) * (NPC // 16)
                    nc.scalar.dma_start(vt_sb[:, vc0:vc1], vT_d[:, vc0:vc1])
                ps = pp1.tile([128, BLK], f32, tag="p1")
                # zero-init: windows overlap, so no single matmul "starts" a
                # column range; zeros make start=False accumulation safe
                # whatever the stale has_written state (0+x == x either way).
                # Emitted before the sel builds so it never gates the MMs.
                nc.vector.memset(ps[:, :cols_blk], 0.0)
                # batched one-hot build on DVE, one op per half; separate
                # tiles so a-MMs don't wait on the b-half's build
                sela = sp.tile([128, max_half * WSPAN], bf16, tag="sela")
                selbt = sp.tile([128, max_half * WSPAN], bf16, tag="selbt")
                for sel_t, h0, hn in ((sela, 0, na_b), (selbt, na_b, nb_b)):
                    if hn == 0:
                        continue
                    in0 = iota_sb[:].rearrange(
                        "p (one w) -> p one w", one=1
                    ).broadcast_to([128, hn, WSPAN])
                    in1 = rel_t[:, h0 : h0 + hn].rearrange(
                        "p (n one) -> p n one", one=1
                    ).broadcast_to([128, hn, WSPAN])
                    outap = sel_t[:, : hn * WSPAN].rearrange(
                        "p (n w) -> p n w", w=WSPAN
                    )
                    nc.vector.tensor_tensor(
                        out=outap, in0=in0, in1=in1,
                        op=mybir.AluOpType.is_equal,
                    )
                # interleave a/b tiles: they target different PE col-groups
                # (psum partitions 0:64 vs 64:128), so adjacent MMs overlap
                inter = []
                for k in range(max(na_b, nb_b)):
                    if k < na_b:
                        inter.append((0, k, blk_a[j][k]))
                    if k < nb_b:
                        inter.append((1, k, blk_b[j][k]))
                for i, (half, tt, c0) in enumerate(inter):
                    r0 = half * 64
                    e16 = et16a if half == 0 else et16b
                    sel_t = sela if half == 0 else selbt
                    w = min(WSPAN, cols_blk - c0)
                    nc.tensor.matmul(
                        out=ps[r0 : r0 + 64, c0 : c0 + w],
                        lhsT=e16[:, tt * 64 : (tt + 1) * 64],
                        rhs=sel_t[:, tt * WSPAN : tt * WSPAN + w],
                        start=False, stop=(i == n_blk - 1),
                        skip_group_check=True,
                    )
                off += n_blk
                x0 = x0p.tile([128, BLK], bf16, tag="x0")
                mid = min(256, cols_blk)
                nc.scalar.copy(x0[:, :mid], ps[:, :mid])
                if cols_blk > mid:
                    nc.scalar.copy(
                        x0[:, mid:cols_blk], ps[:, mid:cols_blk]
                    )
                po_j = emit_mms(j, x0, cols_blk)
                emit_finish(j, po_j, cols_blk)
    _split_multi_waits(nc)
    return nc


def _make_updater(nc, tc, pools, consts):
    pp2, op = pools
    w0_sb, w1_sb, b_sb, vt_sb, outT_d, vt_cols = consts
    f32 = mybir.dt.float32
    bf16 = mybir.dt.bfloat16
    state = {"ot": None, "chunk_col0": 0}

    def emit_mms(j, x0, cols_blk):
        # column-halved with each half's accumulation group opened and
        # closed contiguously; the first half only waits on the first
        # (shorter) half-copy, shrinking the PE stall after the scatter
        po = pp2.tile([128, BLK], f32, tag="p2")
        mid = min(256, cols_blk)
        halves = [(0, mid)] + ([(mid, cols_blk)] if cols_blk > mid else [])
        for h0, h1 in halves:
            nc.tensor.matmul(
                out=po[:, h0:h1], lhsT=w0_sb[:], rhs=x0[:, h0:h1],
                start=True, stop=False,
            )
            nc.tensor.matmul(
                out=po[:, h0:h1],
                lhsT=w1_sb[:],
                rhs=vt_sb[:, j * BLK + h0 : j * BLK + h1],
                start=False, stop=True,
            )
        return po

    def emit_finish(j, po, cols_blk):
        jc = j % OUT_CHUNK
        if jc == 0:
            ot_t = op.tile([128, OUT_CHUNK * BLK], bf16, tag="ot")
            state["ot"] = ot_t
            state["chunk_col0"] = j * BLK
        ot = state["ot"]
        nc.scalar.activation(
            out=ot[:, jc * BLK : jc * BLK + cols_blk],
            in_=po[:, :cols_blk],
            func=mybir.ActivationFunctionType.Identity,
            bias=b_sb[:, 0:1],
            scale=1.0,
        )
        if jc == OUT_CHUNK - 1 or j == N_BLKS - 1:
            chunk_cols = jc * BLK + cols_blk
            nc.scalar.dma_start(
                outT_d[:, state["chunk_col0"] : state["chunk_col0"] + chunk_cols],
                ot[:, :chunk_cols],
            )

    return emit_mms, emit_finish


# ---------------------------------------------------------------------------
# public entry point
# ---------------------------------------------------------------------------


def kernel(vdata, edata_a, edata_b, conn_a, conn_b, W, b, _trace=False):
    in_maps, sched = _preprocess(
        vdata, edata_a, edata_b, conn_a, conn_b, W, b
    )
    nc = _NC_CACHE.get(sched)
    if nc is None:
        nc = _build(sched)
        _NC_CACHE[sched] = nc
    kwargs = {}
    if _trace:
        kwargs = dict(trace=True, trace_cores=[0])
    res = run_bass_kernel_spmd(
        nc, in_maps, core_ids=list(range(N_CORES)), **kwargs
    )

    NTOT = NPC * N_CORES
    out_full = np.empty((NTOT, D_OUT), dtype=np.float32)
    for c in range(N_CORES):
        outT = res.results[c]["outT"].astype(np.float32)  # [128, NPC]
        out_full[c * NPC : (c + 1) * NPC] = outT.T
    out = out_full[:N_NODES]
    if _trace:
        return out, res
    return out
